# revision 7
# baseline (speedup 1.0000x reference)
"""2-layer GAT (PyG GATConv-style, eval mode) on 8 Trainium2 NeuronCores.

Strategy (1D node partitioning, dst-sharded):
  - Nodes are sharded across 8 cores (6250 each). Three SPMD launches:
      L1: h1a = x @ [W1 | W1@A1]  (per-core node shard; A1 folds att vectors)
      L2: layer-1 edge phase (gather by src / segment softmax-sum by dst)
          + bias + ELU + h2pa = elu1 @ [W2 | W2@A2]
      L3: layer-2 edge phase + bias + log_softmax
    The host concatenates per-core shards into full node tables between
    launches (host-mediated halo exchange; not part of HW exec time).
  - Edge phase: per core, its dst nodes are sorted by in-degree (desc) and
    grouped into blocks of 128 (one dst node per partition). Each block's
    edges are padded to the block max degree d_b and laid out [128, d_b].
    One indirect DMA gathers rows [h | a_src] of the full node table by src
    id; a_dst is per-partition; softmax is unnormalized-exp (shift-free is
    numerically safe here) with the denominator reduced alongside.
    Pad slots point at a dummy table row (zeros, a_src = -120) so they
    contribute exactly 0 to messages and ~1e-11 to denominators.
"""

import numpy as np

N = 50000
E = 800000
D_IN = 256
HID = 64
HEADS = 4
OUT = 40
NEG_SLOPE = 0.2

NCORES = 8
NPC = N // NCORES          # 6250 nodes per core
P = 128
NBLK = (NPC + P - 1) // P  # 49 blocks per core
NPAD = NBLK * P            # 6272 slots per core
DUMMY = N                  # dummy row index in node tables
BIG_NEG = -120.0

F1 = HEADS * HID           # 256
C1 = F1 + 2 * HEADS        # 264 = [h1 | a_src1 | a_dst1]
T1 = F1 + HEADS            # 260 table1 row = [h1 | a_src1]
C2 = OUT + 2               # 42  = [h2p | a_src2 | a_dst2]
T2 = OUT + 1               # 41 table2 row = [h2p | a_src2]


def _schedule(src, dst):
    """Per-core degree-sorted block schedule + gather index arrays."""
    core_of = dst // NPC
    dbs_core = np.zeros((NCORES, NBLK), dtype=np.int64)
    per_core = []
    for c in range(NCORES):
        m = core_of == c
        es = src[m].astype(np.int64)
        ed = (dst[m] - c * NPC).astype(np.int64)
        deg = np.bincount(ed, minlength=NPC)
        order = np.argsort(-deg, kind="stable")
        sidx = np.argsort(ed, kind="stable")
        es_sorted = es[sidx]
        starts = np.zeros(NPC + 1, dtype=np.int64)
        np.cumsum(deg, out=starts[1:])
        for b in range(NBLK):
            i = b * P
            dbs_core[c, b] = deg[order[i]] if i < NPC else 0
        per_core.append((deg, order, es_sorted, starts))

    dbs = np.maximum(dbs_core.max(axis=0), 1).astype(np.int64)
    offs = np.zeros(NBLK + 1, dtype=np.int64)
    np.cumsum(dbs, out=offs[1:])
    totd = int(offs[-1])

    idx_arrs = []
    node_of = []
    for c in range(NCORES):
        deg, order, es_sorted, starts = per_core[c]
        idx = np.full((P, totd), DUMMY, dtype=np.int32)
        nof = np.full(NPAD, -1, dtype=np.int64)
        for b in range(NBLK):
            o = offs[b]
            for p in range(P):
                i = b * P + p
                if i >= NPC:
                    break
                node = order[i]
                nof[i] = node
                d = deg[node]
                if d:
                    idx[p, o:o + d] = es_sorted[starts[node]:starts[node] + d]
        idx_arrs.append(idx)
        node_of.append(nof)
    return dbs, offs, totd, idx_arrs, node_of


def _slots(arr_128xnblkw, w):
    """[128, NBLK*w] core output -> [NPAD, w] slot-major rows."""
    return (
        arr_128xnblkw.reshape(P, NBLK, w).transpose(1, 0, 2).reshape(NPAD, w)
    )


def _build_l1(mybir, bacc, tile, bass):
    f32 = mybir.dt.float32
    nc = bacc.Bacc("TRN2", target_bir_lowering=False, debug=False,
                   num_devices=NCORES)
    xT = nc.dram_tensor("xT", [P, 2, NPAD], f32, kind="ExternalInput")
    W1b = nc.dram_tensor("W1b", [P, 2 * C1], f32, kind="ExternalInput")
    h1a = nc.dram_tensor("h1a", [P, NBLK * C1], f32, kind="ExternalOutput")
    with tile.TileContext(nc) as tc:
        with (
            tc.tile_pool(name="const", bufs=1) as cpool,
            tc.tile_pool(name="ps", bufs=4, space="PSUM") as pspool,
            tc.tile_pool(name="ev", bufs=3) as evpool,
        ):
            xT_sb = cpool.tile([P, 2, NPAD], f32)
            nc.sync.dma_start(out=xT_sb[:], in_=xT[:])
            W1b_sb = cpool.tile([P, 2 * C1], f32)
            nc.sync.dma_start(out=W1b_sb[:], in_=W1b[:])
            for b in range(NBLK):
                ps = pspool.tile([P, C1], f32)
                nc.tensor.matmul(ps[:], lhsT=xT_sb[:, 0, b * P:(b + 1) * P],
                                 rhs=W1b_sb[:, 0:C1], start=True, stop=False)
                nc.tensor.matmul(ps[:], lhsT=xT_sb[:, 1, b * P:(b + 1) * P],
                                 rhs=W1b_sb[:, C1:2 * C1], start=False,
                                 stop=True)
                ev = evpool.tile([P, C1], f32)
                nc.vector.tensor_copy(ev[:], ps[:])
                nc.sync.dma_start(out=h1a[:, b * C1:(b + 1) * C1], in_=ev[:])
    nc.compile()
    return nc


def _build_edge(mybir, bacc, tile, bass, dbs, offs, totd, *, nfeat, nhead,
                table_rows, make_dense_tail):
    """Edge-phase program builder shared by L2 and L3.

    nfeat: message width (256 / 40); nhead: 4 / 1.
    The per-edge row stream gst[p, j, :] = [msg(nfeat) | a_src(nhead)] is
    host-pre-gathered (this bedrock image ships no HIPI GPSIMD ucode, so
    dma_gather / indirect DMA are unavailable on device); a_dst input is
    per (block, partition).
    make_dense_tail(nc, tc, pools, b, node_out):
        node_out = [P, nfeat] post-aggregation (bias added) -> emits the
        rest + output DMA for block b.
    """
    f32 = mybir.dt.float32
    trow = nfeat + nhead
    nc = bacc.Bacc("TRN2", target_bir_lowering=False, debug=False,
                   num_devices=NCORES)
    gst = nc.dram_tensor("gst", [P, totd, trow], f32, kind="ExternalInput")
    adst = nc.dram_tensor("adst", [P, NBLK * nhead], f32,
                          kind="ExternalInput")
    biast = nc.dram_tensor("bias", [P, nfeat], f32, kind="ExternalInput")
    tail_inputs = {}

    with tile.TileContext(nc) as tc:
        with (
            tc.tile_pool(name="const", bufs=1) as cpool,
            tc.tile_pool(name="g", bufs=2) as gpool,
            tc.tile_pool(name="s", bufs=2) as spool,
            tc.tile_pool(name="z", bufs=2) as zpool,
            tc.tile_pool(name="nsm", bufs=2) as npool,
            tc.tile_pool(name="ps", bufs=4, space="PSUM") as pspool,
        ):
            pools = (cpool, gpool, spool, zpool, npool, pspool)
            adst_sb = cpool.tile([P, NBLK * nhead], f32)
            nc.sync.dma_start(out=adst_sb[:], in_=adst[:])
            bias_sb = cpool.tile([P, nfeat], f32)
            nc.sync.dma_start(out=bias_sb[:], in_=biast[:])
            tail_state = make_dense_tail(nc, tc, pools, None, None)

            for b in range(NBLK):
                db = int(dbs[b])
                o = int(offs[b])
                G = gpool.tile([P, db, trow], f32, tag="G")
                nc.sync.dma_start(out=G[:], in_=gst[:, o:o + db, :])
                # z = a_src[src] + a_dst[dst];  w = exp(leaky_relu(z))
                zl = zpool.tile([P, db, nhead], f32, tag="zl")
                a_view = (adst_sb[:, b * nhead:(b + 1) * nhead]
                          .unsqueeze(1).broadcast_to([P, db, nhead]))
                nc.vector.tensor_tensor(zl[:], G[:, :, nfeat:trow], a_view,
                                        op=mybir.AluOpType.add)
                zr = zpool.tile([P, db, nhead], f32, tag="zr")
                nc.vector.scalar_tensor_tensor(
                    zr[:], in0=zl[:], scalar=NEG_SLOPE, in1=zl[:],
                    op0=mybir.AluOpType.mult, op1=mybir.AluOpType.max)
                w = zpool.tile([P, db, nhead], f32, tag="w")
                nc.scalar.activation(w[:], zr[:],
                                     mybir.ActivationFunctionType.Exp)
                # S = G_msg * w (broadcast over channels of each head)
                S = spool.tile([P, db, nfeat], f32, tag="S")
                ch = nfeat // nhead
                g4 = G[:, :, 0:nfeat].rearrange("p j (h c) -> p j h c",
                                                h=nhead)
                s4 = S[:].rearrange("p j (h c) -> p j h c", h=nhead)
                w4 = w[:].unsqueeze(3).broadcast_to([P, db, nhead, ch])
                nc.vector.tensor_tensor(s4, g4, w4, op=mybir.AluOpType.mult)
                # segment sums over the block's edges
                msum = npool.tile([P, nfeat], f32, tag="msum")
                nc.vector.tensor_reduce(
                    msum[:], S[:].rearrange("p j c -> p c j"),
                    axis=mybir.AxisListType.X, op=mybir.AluOpType.add)
                ws = npool.tile([P, nhead], f32, tag="ws")
                nc.vector.tensor_reduce(
                    ws[:], w[:].rearrange("p j h -> p h j"),
                    axis=mybir.AxisListType.X, op=mybir.AluOpType.add)
                rws = npool.tile([P, nhead], f32, tag="rws")
                nc.vector.reciprocal(rws[:], ws[:])
                o1 = npool.tile([P, nfeat], f32, tag="o1")
                nc.vector.tensor_tensor(
                    o1[:].rearrange("p (h c) -> p h c", h=nhead),
                    msum[:].rearrange("p (h c) -> p h c", h=nhead),
                    rws[:].unsqueeze(2).broadcast_to([P, nhead, ch]),
                    op=mybir.AluOpType.mult)
                o1b = npool.tile([P, nfeat], f32, tag="o1b")
                nc.vector.tensor_tensor(o1b[:], o1[:], bias_sb[:],
                                        op=mybir.AluOpType.add)
                make_dense_tail(nc, tc, pools, b, (o1b, tail_state))
    nc.compile()
    return nc, tail_inputs


def _build_l2(mybir, bacc, tile, bass, dbs, offs, totd):
    f32 = mybir.dt.float32
    from concourse.masks import make_identity

    holder = {}

    def tail(nc, tc, pools, b, arg):
        cpool, gpool, spool, zpool, npool, pspool = pools
        if b is None:
            # one-time setup: identity, W2b, output tensor
            W2b = nc.dram_tensor("W2b", [P, 2 * C2], f32,
                                 kind="ExternalInput")
            h2pa = nc.dram_tensor("h2pa", [P, NBLK * C2], f32,
                                  kind="ExternalOutput")
            W2b_sb = cpool.tile([P, 2 * C2], f32)
            nc.sync.dma_start(out=W2b_sb[:], in_=W2b[:])
            ident = cpool.tile([P, P], f32)
            make_identity(nc, ident[:])
            holder["st"] = (W2b_sb, ident, h2pa)
            return holder["st"]
        o1b, (W2b_sb, ident, h2pa) = arg
        # elu(x) = max(x, exp(min(x, 0)) - 1)
        m0 = npool.tile([P, F1], f32, tag="m0")
        nc.vector.tensor_scalar(m0[:], in0=o1b[:], scalar1=0.0, scalar2=None,
                                op0=mybir.AluOpType.min)
        u = npool.tile([P, F1], f32, tag="u")
        nc.scalar.activation(u[:], m0[:], mybir.ActivationFunctionType.Exp)
        elu = npool.tile([P, F1], f32, tag="elu")
        nc.vector.scalar_tensor_tensor(
            elu[:], in0=u[:], scalar=-1.0, in1=o1b[:],
            op0=mybir.AluOpType.add, op1=mybir.AluOpType.max)
        # transpose elu -> [feat, node] for the dense tail matmul
        eT = []
        for k in range(2):
            psT = pspool.tile([P, P], f32, tag="psT")
            nc.tensor.transpose(psT[:], elu[:, k * P:(k + 1) * P], ident[:])
            eTk = npool.tile([P, P], f32, tag=f"eT{k}")
            nc.vector.tensor_copy(eTk[:], psT[:])
            eT.append(eTk)
        psC = pspool.tile([P, C2], f32, tag="psC")
        nc.tensor.matmul(psC[:], lhsT=eT[0][:], rhs=W2b_sb[:, 0:C2],
                         start=True, stop=False)
        nc.tensor.matmul(psC[:], lhsT=eT[1][:], rhs=W2b_sb[:, C2:2 * C2],
                         start=False, stop=True)
        hout = npool.tile([P, C2], f32, tag="hout")
        nc.vector.tensor_copy(hout[:], psC[:])
        nc.sync.dma_start(out=h2pa[:, b * C2:(b + 1) * C2], in_=hout[:])

    nc, _ = _build_edge(mybir, bacc, tile, bass, dbs, offs, totd,
                        nfeat=F1, nhead=HEADS, table_rows=N + 1,
                        make_dense_tail=tail)
    return nc


def _build_l3(mybir, bacc, tile, bass, dbs, offs, totd):
    f32 = mybir.dt.float32
    holder = {}

    def tail(nc, tc, pools, b, arg):
        cpool, gpool, spool, zpool, npool, pspool = pools
        if b is None:
            res = nc.dram_tensor("res", [P, NBLK * OUT], f32,
                                 kind="ExternalOutput")
            holder["res"] = res
            return res
        o2b, res = arg
        # log_softmax over the 40 classes
        m = npool.tile([P, 1], f32, tag="m")
        nc.vector.tensor_reduce(m[:], o2b[:], axis=mybir.AxisListType.X,
                                op=mybir.AluOpType.max)
        negm = npool.tile([P, 1], f32, tag="negm")
        nc.vector.tensor_scalar(negm[:], in0=m[:], scalar1=-1.0, scalar2=None,
                                op0=mybir.AluOpType.mult)
        t = npool.tile([P, OUT], f32, tag="t")
        nc.scalar.activation(t[:], o2b[:], mybir.ActivationFunctionType.Exp,
                             bias=negm[:], scale=1.0)
        s = npool.tile([P, 1], f32, tag="s")
        nc.vector.tensor_reduce(s[:], t[:], axis=mybir.AxisListType.X,
                                op=mybir.AluOpType.add)
        ls = npool.tile([P, 1], f32, tag="ls")
        nc.scalar.activation(ls[:], s[:], mybir.ActivationFunctionType.Ln)
        resb = npool.tile([P, OUT], f32, tag="resb")
        nc.vector.scalar_tensor_tensor(
            resb[:], in0=o2b[:], scalar=negm[:], in1=ls[:].broadcast_to([P, OUT]),
            op0=mybir.AluOpType.add, op1=mybir.AluOpType.subtract)
        nc.sync.dma_start(out=res[:, b * OUT:(b + 1) * OUT], in_=resb[:])

    nc, _ = _build_edge(mybir, bacc, tile, bass, dbs, offs, totd,
                        nfeat=OUT, nhead=1, table_rows=N + 1,
                        make_dense_tail=tail)
    return nc


def _run(nc, in_maps, trace=False):
    from concourse import bass_utils
    return bass_utils.run_bass_kernel_spmd(
        nc, in_maps, core_ids=list(range(NCORES)), trace=trace)


def kernel(x, edge_index, W1, att_src1, att_dst1, b1, W2, att_src2, att_dst2,
           b2, _profile=None):
    import concourse.bacc as bacc
    import concourse.bass as bass
    import concourse.mybir as mybir
    import concourse.tile as tile

    x = np.asarray(x, dtype=np.float32)
    ei = np.asarray(edge_index, dtype=np.int64)
    W1 = np.asarray(W1, dtype=np.float32)
    att_src1 = np.asarray(att_src1, dtype=np.float32)
    att_dst1 = np.asarray(att_dst1, dtype=np.float32)
    b1 = np.asarray(b1, dtype=np.float32)
    W2 = np.asarray(W2, dtype=np.float32)
    att_src2 = np.asarray(att_src2, dtype=np.float32)
    att_dst2 = np.asarray(att_dst2, dtype=np.float32)
    b2 = np.asarray(b2, dtype=np.float32)

    # ---- host prep: weights ------------------------------------------------
    # A1 maps h1 -> [a_src1 | a_dst1] (einsum 'nhc,hc->nh' per head)
    A1 = np.zeros((F1, 2 * HEADS), dtype=np.float32)
    for h in range(HEADS):
        A1[h * HID:(h + 1) * HID, h] = att_src1[h]
        A1[h * HID:(h + 1) * HID, HEADS + h] = att_dst1[h]
    W1b = np.concatenate([W1, W1 @ A1], axis=1)          # [256, 264]
    A2 = np.zeros((OUT, 2), dtype=np.float32)
    A2[:, 0] = att_src2[0]
    A2[:, 1] = att_dst2[0]
    W2b = np.concatenate([W2, W2 @ A2], axis=1)          # [256, 42]

    # ---- host prep: graph schedule ----------------------------------------
    loops = np.arange(N, dtype=np.int64)
    src = np.concatenate([ei[0], loops])
    dst = np.concatenate([ei[1], loops])
    dbs, offs, totd, idx_arrs, node_of = _schedule(src, dst)

    # ---- L1: h1a = x @ W1b (node-sharded) ---------------------------------
    nc1 = _build_l1(mybir, bacc, tile, bass)
    W1b_packed = np.concatenate([W1b[0:P], W1b[P:2 * P]], axis=1)  # [128,528]
    in_maps1 = []
    for c in range(NCORES):
        xs = np.zeros((P, 2, NPAD), dtype=np.float32)
        xc = x[c * NPC:(c + 1) * NPC]                    # [6250, 256]
        xt = np.ascontiguousarray(xc.T)                  # [256, 6250]
        xs[:, 0, :NPC] = xt[0:P]
        xs[:, 1, :NPC] = xt[P:2 * P]
        in_maps1.append({"xT": xs, "W1b": W1b_packed})
    res1 = _run(nc1, in_maps1, trace=_profile is not None)
    if _profile is not None and res1.exec_time_ns:
        _profile.append(("L1", res1.exec_time_ns))

    # assemble full node table for layer-1 edge phase
    table1 = np.zeros((N + 1, T1), dtype=np.float32)
    adst_all = np.zeros((N, HEADS), dtype=np.float32)
    for c in range(NCORES):
        slots = _slots(res1.results[c]["h1a"], C1)       # [NPAD, 264]
        table1[c * NPC:(c + 1) * NPC] = slots[:NPC, :T1]
        adst_all[c * NPC:(c + 1) * NPC] = slots[:NPC, T1:C1]
    table1[DUMMY, F1:T1] = BIG_NEG

    # ---- L2: layer-1 edge phase + ELU + dense -----------------------------
    nc2 = _build_l2(mybir, bacc, tile, bass, dbs, offs, totd)
    W2b_packed = np.concatenate([W2b[0:P], W2b[P:2 * P]], axis=1)  # [128, 84]
    bias1 = np.tile(b1.reshape(1, F1), (P, 1)).astype(np.float32)
    in_maps2 = []
    for c in range(NCORES):
        ad = np.zeros((P, NBLK * HEADS), dtype=np.float32)
        nof = node_of[c]
        for b in range(NBLK):
            valid = nof[b * P:(b + 1) * P]
            vm = valid >= 0
            ad_blk = np.zeros((P, HEADS), dtype=np.float32)
            ad_blk[vm] = adst_all[c * NPC + valid[vm]]
            ad[:, b * HEADS:(b + 1) * HEADS] = ad_blk
        in_maps2.append({"gst": table1[idx_arrs[c]], "adst": ad,
                         "bias": bias1, "W2b": W2b_packed})
    res2 = _run(nc2, in_maps2, trace=_profile is not None)
    if _profile is not None and res2.exec_time_ns:
        _profile.append(("L2", res2.exec_time_ns))

    # assemble layer-2 node table
    table2 = np.zeros((N + 1, T2), dtype=np.float32)
    adst2_pc = []
    for c in range(NCORES):
        h2pa = res2.results[c]["h2pa"]                   # [128, NBLK*42]
        slots = _slots(h2pa, C2)                         # [NPAD, 42]
        nof = node_of[c]
        vm = nof >= 0
        table2[c * NPC + nof[vm]] = slots[vm][:, :T2]
        adst2_pc.append(np.ascontiguousarray(h2pa[:, T2::C2]))  # [128, NBLK]
    table2[DUMMY, OUT:T2] = BIG_NEG

    # ---- L3: layer-2 edge phase + log_softmax -----------------------------
    nc3 = _build_l3(mybir, bacc, tile, bass, dbs, offs, totd)
    bias2 = np.tile(b2.reshape(1, OUT), (P, 1)).astype(np.float32)
    in_maps3 = []
    for c in range(NCORES):
        in_maps3.append({"gst": table2[idx_arrs[c]],
                         "adst": adst2_pc[c], "bias": bias2})
    res3 = _run(nc3, in_maps3, trace=_profile is not None)
    if _profile is not None and res3.exec_time_ns:
        _profile.append(("L3", res3.exec_time_ns))

    out = np.zeros((N, OUT), dtype=np.float32)
    for c in range(NCORES):
        slots = _slots(res3.results[c]["res"], OUT)      # [NPAD, 40]
        nof = node_of[c]
        vm = nof >= 0
        out[c * NPC + nof[vm]] = slots[vm]
    return out


# revision 12
# speedup vs baseline: 1.4226x; 1.4226x over previous
"""2-layer GAT (PyG GATConv-style, eval mode) on 8 Trainium2 NeuronCores.

Strategy (1D node partitioning, dst-sharded):
  - Nodes are sharded across 8 cores (6250 each). Three SPMD launches:
      L1: h1a = x @ [W1 | W1@A1]  (per-core node shard; A1 folds att vectors)
      L2: layer-1 edge phase (segment softmax-sum by dst) + bias + ELU
          + h2pa = elu1 @ [W2 | W2@A2]
      L3: layer-2 edge phase + bias + log_softmax
    The host concatenates per-core shards into full node tables between
    launches and pre-expands the per-edge-slot row stream (host-mediated
    halo exchange / gather; this bedrock image ships no HIPI GPSIMD ucode,
    so dma_gather / indirect DMA are unavailable on device).
  - Edge phase: per core, its dst nodes are sorted by in-degree (desc) and
    grouped into blocks of 128 (one dst node per partition). Each block's
    edges are padded to the block max degree d_b and laid out [128, d_b]
    along the free dimension; the row stream entry is [h | a_src] in fp16.
    a_dst is per-partition. Softmax uses a constant -10 logit shift (exact:
    softmax is shift invariant; keeps exp() in fp16 range); the denominator
    is reduced from the same fp16 weights (+1e-20 so empty pad nodes stay
    finite). Pad slots reference a dummy table row (h=0, a_src=-120) so
    they contribute exactly 0.
  - Per block: DMA the fp16 rows; z/lrelu on DVE; exp is computed on the
    ScalarEngine directly into a channel-expanded [128, d_b, H, C] buffer
    (broadcast read AP), so the message scaling is a unit-stride fp16
    tensor_tensor at 2x packing; the segment sum is an in-place log-tree
    of fp16 adds (fp32 final level).
"""

import numpy as np

N = 50000
E = 800000
D_IN = 256
HID = 64
HEADS = 4
OUT = 40
NEG_SLOPE = 0.2

NCORES = 8
NPC = N // NCORES          # 6250 nodes per core
P = 128
NBLK = (NPC + P - 1) // P  # 49 blocks per core
NPAD = NBLK * P            # 6272 slots per core
DUMMY = N                  # dummy row index in node tables
BIG_NEG = -120.0
SHIFT = 10.0               # constant logit shift before exp (fp16 range)

F1 = HEADS * HID           # 256
C1 = F1 + 2 * HEADS        # 264 = [h1 | a_src1 | a_dst1]
T1 = F1 + HEADS            # 260 table1 row = [h1 | a_src1]
C2 = OUT + 2               # 42  = [h2p | a_src2 | a_dst2]
T2 = OUT + 1               # 41 table2 row = [h2p | a_src2]


def _schedule(src, dst):
    """Per-core degree-sorted block schedule + gather index arrays."""
    core_of = dst // NPC
    dbs_core = np.zeros((NCORES, NBLK), dtype=np.int64)
    per_core = []
    for c in range(NCORES):
        m = core_of == c
        es = src[m].astype(np.int64)
        ed = (dst[m] - c * NPC).astype(np.int64)
        deg = np.bincount(ed, minlength=NPC)
        order = np.argsort(-deg, kind="stable")
        sidx = np.argsort(ed, kind="stable")
        es_sorted = es[sidx]
        starts = np.zeros(NPC + 1, dtype=np.int64)
        np.cumsum(deg, out=starts[1:])
        for b in range(NBLK):
            i = b * P
            dbs_core[c, b] = deg[order[i]] if i < NPC else 0
        per_core.append((deg, order, es_sorted, starts))

    dbs = np.maximum(dbs_core.max(axis=0), 1).astype(np.int64)
    offs = np.zeros(NBLK + 1, dtype=np.int64)
    np.cumsum(dbs, out=offs[1:])
    totd = int(offs[-1])

    idx_arrs = []
    node_of = []
    for c in range(NCORES):
        deg, order, es_sorted, starts = per_core[c]
        idx = np.full((P, totd), DUMMY, dtype=np.int32)
        nof = np.full(NPAD, -1, dtype=np.int64)
        for b in range(NBLK):
            o = offs[b]
            for p in range(P):
                i = b * P + p
                if i >= NPC:
                    break
                node = order[i]
                nof[i] = node
                d = deg[node]
                if d:
                    idx[p, o:o + d] = es_sorted[starts[node]:starts[node] + d]
        idx_arrs.append(idx)
        node_of.append(nof)
    return dbs, offs, totd, idx_arrs, node_of


def _slots(arr_128xnblkw, w):
    """[128, NBLK*w] core output -> [NPAD, w] slot-major rows."""
    return (
        arr_128xnblkw.reshape(P, NBLK, w).transpose(1, 0, 2).reshape(NPAD, w)
    )


def _build_l1(mybir, bacc, tile, bass):
    f32 = mybir.dt.float32
    f32r = mybir.dt.float32r
    nc = bacc.Bacc("TRN2", target_bir_lowering=False, debug=False,
                   num_devices=NCORES)
    xT = nc.dram_tensor("xT", [P, 2, NPAD], f32r, kind="ExternalInput")
    W1b = nc.dram_tensor("W1b", [P, 2 * C1], f32r, kind="ExternalInput")
    h1a = nc.dram_tensor("h1a", [P, NBLK * C1], f32, kind="ExternalOutput")
    with tile.TileContext(nc) as tc:
        with (
            tc.tile_pool(name="const", bufs=1) as cpool,
            tc.tile_pool(name="x", bufs=3) as xpool,
            tc.tile_pool(name="ps", bufs=4, space="PSUM") as pspool,
            tc.tile_pool(name="ev", bufs=3) as evpool,
        ):
            W1b_sb = cpool.tile([P, 2 * C1], f32r)
            nc.sync.dma_start(out=W1b_sb[:], in_=W1b[:])
            for b in range(NBLK):
                xt = xpool.tile([P, 2, P], f32r, tag="xt")
                nc.sync.dma_start(out=xt[:], in_=xT[:, :, b * P:(b + 1) * P])
                ps = pspool.tile([P, C1], f32)
                nc.tensor.matmul(ps[:], lhsT=xt[:, 0, :],
                                 rhs=W1b_sb[:, 0:C1],
                                 start=True, stop=False)
                nc.tensor.matmul(ps[:], lhsT=xt[:, 1, :],
                                 rhs=W1b_sb[:, C1:2 * C1],
                                 start=False, stop=True)
                ev = evpool.tile([P, C1], f32)
                if b % 2 == 0:
                    nc.vector.tensor_copy(ev[:], ps[:])
                else:
                    nc.scalar.copy(ev[:], ps[:])
                nc.sync.dma_start(out=h1a[:, b * C1:(b + 1) * C1], in_=ev[:])
    nc.compile()
    return nc


def _tree_reduce(nc, mybir, G, msum, db, nfeat):
    """In-place fp16 log-tree sum over G[:, 0:db, 0:nfeat] -> msum (fp32)."""
    add = mybir.AluOpType.add
    if db == 1:
        nc.vector.tensor_copy(msum[:], G[:, 0, 0:nfeat])
        return
    cur = db
    while True:
        if cur % 2 == 1:
            nc.vector.tensor_tensor(G[:, 0:1, 0:nfeat], G[:, 0:1, 0:nfeat],
                                    G[:, cur - 1:cur, 0:nfeat], op=add)
            cur -= 1
        if cur == 2:
            nc.vector.tensor_tensor(msum[:], G[:, 0, 0:nfeat],
                                    G[:, 1, 0:nfeat], op=add)
            return
        h = cur // 2
        nc.vector.tensor_tensor(G[:, 0:h, 0:nfeat], G[:, 0:h, 0:nfeat],
                                G[:, h:cur, 0:nfeat], op=add)
        cur = h


def _build_edge(mybir, bacc, tile, bass, dbs, offs, totd, *, nfeat, nhead,
                make_dense_tail):
    """Edge-phase program builder shared by L2 and L3 (see module doc)."""
    f32 = mybir.dt.float32
    f16 = mybir.dt.float16
    trow = nfeat + nhead
    ch = nfeat // nhead
    nc = bacc.Bacc("TRN2", target_bir_lowering=False, debug=False,
                   num_devices=NCORES)
    gst = nc.dram_tensor("gst", [P, totd, trow], f16, kind="ExternalInput")
    adst = nc.dram_tensor("adst", [P, NBLK * nhead], f32,
                          kind="ExternalInput")
    biast = nc.dram_tensor("bias", [P, nfeat], f32, kind="ExternalInput")

    with tile.TileContext(nc) as tc:
        with (
            tc.tile_pool(name="const", bufs=1) as cpool,
            tc.tile_pool(name="g", bufs=3) as gpool,
            tc.tile_pool(name="w", bufs=2) as wpool,
            tc.tile_pool(name="z", bufs=2) as zpool,
            tc.tile_pool(name="nsm", bufs=2) as npool,
            tc.tile_pool(name="ps", bufs=2, space="PSUM") as pspool,
        ):
            pools = (cpool, gpool, wpool, zpool, npool, pspool)
            adst_sb = cpool.tile([P, NBLK * nhead], f32)
            nc.sync.dma_start(out=adst_sb[:], in_=adst[:])
            bias_sb = cpool.tile([P, nfeat], f32)
            nc.sync.dma_start(out=bias_sb[:], in_=biast[:])
            shift_sb = cpool.tile([P, 1], f32)
            nc.vector.memset(shift_sb[:], -SHIFT)
            tail_state = make_dense_tail(nc, tc, pools, None, None)

            for b in range(NBLK):
                db = int(dbs[b])
                o = int(offs[b])
                G = gpool.tile([P, db, trow], f16, tag="G")
                nc.sync.dma_start(out=G[:], in_=gst[:, o:o + db, :])
                # z = a_src[src] + a_dst[dst]; zr = leaky_relu(z)
                zl = zpool.tile([P, db, nhead], f32, tag="zl")
                a_view = (adst_sb[:, b * nhead:(b + 1) * nhead]
                          .unsqueeze(1).broadcast_to([P, db, nhead]))
                nc.vector.tensor_tensor(zl[:], G[:, :, nfeat:trow], a_view,
                                        op=mybir.AluOpType.add)
                zr = zpool.tile([P, db, nhead], f32, tag="zr")
                nc.vector.scalar_tensor_tensor(
                    zr[:], in0=zl[:], scalar=NEG_SLOPE, in1=zl[:],
                    op0=mybir.AluOpType.mult, op1=mybir.AluOpType.max)
                # w = exp(zr - SHIFT), written channel-expanded in fp16 (ACT)
                w64 = wpool.tile([P, db, nhead, ch], f16, tag="w64")
                nc.scalar.activation(
                    w64[:], zr[:].unsqueeze(3).broadcast_to([P, db, nhead, ch]),
                    mybir.ActivationFunctionType.Exp, bias=shift_sb[:],
                    scale=1.0)
                # denominators from the same fp16 weights (+eps for pad rows)
                ws = npool.tile([P, nhead], f32, tag="ws")
                nc.vector.tensor_reduce(
                    ws[:], w64[:, :, :, 0].rearrange("p j h -> p h j"),
                    axis=mybir.AxisListType.X, op=mybir.AluOpType.add)
                nc.vector.tensor_scalar(ws[:], in0=ws[:], scalar1=1e-20,
                                        scalar2=None, op0=mybir.AluOpType.add)
                rws = npool.tile([P, nhead], f32, tag="rws")
                nc.vector.reciprocal(rws[:], ws[:])
                # messages scaled in place (fp16, unit-stride 2x) + tree sum
                g4 = G[:, :, 0:nfeat].rearrange("p j (h c) -> p j h c",
                                                h=nhead)
                nc.vector.tensor_tensor(g4, g4, w64[:],
                                        op=mybir.AluOpType.mult)
                msum = npool.tile([P, nfeat], f32, tag="msum")
                _tree_reduce(nc, mybir, G, msum, db, nfeat)
                # normalize + bias
                o1 = npool.tile([P, nfeat], f32, tag="o1")
                nc.vector.tensor_tensor(
                    o1[:].rearrange("p (h c) -> p h c", h=nhead),
                    msum[:].rearrange("p (h c) -> p h c", h=nhead),
                    rws[:].unsqueeze(2).broadcast_to([P, nhead, ch]),
                    op=mybir.AluOpType.mult)
                o1b = npool.tile([P, nfeat], f32, tag="o1b")
                nc.vector.tensor_tensor(o1b[:], o1[:], bias_sb[:],
                                        op=mybir.AluOpType.add)
                make_dense_tail(nc, tc, pools, b, (o1b, tail_state))
    nc.compile()
    return nc


def _build_l2(mybir, bacc, tile, bass, dbs, offs, totd):
    f32 = mybir.dt.float32
    f16 = mybir.dt.float16
    from concourse.masks import make_identity

    holder = {}

    def tail(nc, tc, pools, b, arg):
        cpool, gpool, wpool, zpool, npool, pspool = pools
        if b is None:
            W2b = nc.dram_tensor("W2b", [P, 2 * C2], f16,
                                 kind="ExternalInput")
            h2pa = nc.dram_tensor("h2pa", [P, NBLK * C2], f32,
                                  kind="ExternalOutput")
            W2b_sb = cpool.tile([P, 2 * C2], f16)
            nc.sync.dma_start(out=W2b_sb[:], in_=W2b[:])
            ident = cpool.tile([P, P], f16)
            make_identity(nc, ident[:])
            holder["st"] = (W2b_sb, ident, h2pa)
            return holder["st"]
        o1b, (W2b_sb, ident, h2pa) = arg
        # elu(x) = max(x, exp(min(x, 0)) - 1), emitted in fp16
        m0 = npool.tile([P, F1], f32, tag="m0")
        nc.vector.tensor_scalar(m0[:], in0=o1b[:], scalar1=0.0, scalar2=None,
                                op0=mybir.AluOpType.min)
        u = npool.tile([P, F1], f32, tag="u")
        nc.scalar.activation(u[:], m0[:], mybir.ActivationFunctionType.Exp)
        elu = npool.tile([P, F1], f16, tag="elu")
        nc.vector.scalar_tensor_tensor(
            elu[:], in0=u[:], scalar=-1.0, in1=o1b[:],
            op0=mybir.AluOpType.add, op1=mybir.AluOpType.max)
        # transpose elu -> [feat, node] for the dense tail matmul
        eT = []
        for k in range(2):
            psT = pspool.tile([P, P], f16, tag="psT")
            nc.tensor.transpose(psT[:], elu[:, k * P:(k + 1) * P], ident[:])
            eTk = npool.tile([P, P], f16, tag=f"eT{k}")
            nc.vector.tensor_copy(eTk[:], psT[:])
            eT.append(eTk)
        psC = pspool.tile([P, C2], f32, tag="psC")
        nc.tensor.matmul(psC[:], lhsT=eT[0][:], rhs=W2b_sb[:, 0:C2],
                         start=True, stop=False)
        nc.tensor.matmul(psC[:], lhsT=eT[1][:], rhs=W2b_sb[:, C2:2 * C2],
                         start=False, stop=True)
        hout = npool.tile([P, C2], f32, tag="hout")
        nc.scalar.copy(hout[:], psC[:])
        nc.sync.dma_start(out=h2pa[:, b * C2:(b + 1) * C2], in_=hout[:])

    return _build_edge(mybir, bacc, tile, bass, dbs, offs, totd,
                       nfeat=F1, nhead=HEADS, make_dense_tail=tail)


def _build_l3(mybir, bacc, tile, bass, dbs, offs, totd):
    f32 = mybir.dt.float32
    holder = {}

    def tail(nc, tc, pools, b, arg):
        cpool, gpool, wpool, zpool, npool, pspool = pools
        if b is None:
            res = nc.dram_tensor("res", [P, NBLK * OUT], f32,
                                 kind="ExternalOutput")
            holder["res"] = res
            return res
        o2b, res = arg
        # log_softmax over the 40 classes
        m = npool.tile([P, 1], f32, tag="m")
        nc.vector.tensor_reduce(m[:], o2b[:], axis=mybir.AxisListType.X,
                                op=mybir.AluOpType.max)
        negm = npool.tile([P, 1], f32, tag="negm")
        nc.vector.tensor_scalar(negm[:], in0=m[:], scalar1=-1.0, scalar2=None,
                                op0=mybir.AluOpType.mult)
        t = npool.tile([P, OUT], f32, tag="t")
        nc.scalar.activation(t[:], o2b[:], mybir.ActivationFunctionType.Exp,
                             bias=negm[:], scale=1.0)
        s = npool.tile([P, 1], f32, tag="s")
        nc.vector.tensor_reduce(s[:], t[:], axis=mybir.AxisListType.X,
                                op=mybir.AluOpType.add)
        ls = npool.tile([P, 1], f32, tag="ls")
        nc.scalar.activation(ls[:], s[:], mybir.ActivationFunctionType.Ln)
        resb = npool.tile([P, OUT], f32, tag="resb")
        nc.vector.scalar_tensor_tensor(
            resb[:], in0=o2b[:], scalar=negm[:],
            in1=ls[:].broadcast_to([P, OUT]),
            op0=mybir.AluOpType.add, op1=mybir.AluOpType.subtract)
        nc.sync.dma_start(out=res[:, b * OUT:(b + 1) * OUT], in_=resb[:])

    return _build_edge(mybir, bacc, tile, bass, dbs, offs, totd,
                       nfeat=OUT, nhead=1, make_dense_tail=tail)


def _run(nc, in_maps, trace=False):
    from concourse import bass_utils
    return bass_utils.run_bass_kernel_spmd(
        nc, in_maps, core_ids=list(range(NCORES)), trace=trace)


def kernel(x, edge_index, W1, att_src1, att_dst1, b1, W2, att_src2, att_dst2,
           b2, _profile=None):
    import concourse.bacc as bacc
    import concourse.bass as bass
    import concourse.mybir as mybir
    import concourse.tile as tile

    x = np.asarray(x, dtype=np.float32)
    ei = np.asarray(edge_index, dtype=np.int64)
    W1 = np.asarray(W1, dtype=np.float32)
    att_src1 = np.asarray(att_src1, dtype=np.float32)
    att_dst1 = np.asarray(att_dst1, dtype=np.float32)
    b1 = np.asarray(b1, dtype=np.float32)
    W2 = np.asarray(W2, dtype=np.float32)
    att_src2 = np.asarray(att_src2, dtype=np.float32)
    att_dst2 = np.asarray(att_dst2, dtype=np.float32)
    b2 = np.asarray(b2, dtype=np.float32)

    # ---- host prep: weights ------------------------------------------------
    A1 = np.zeros((F1, 2 * HEADS), dtype=np.float32)
    for h in range(HEADS):
        A1[h * HID:(h + 1) * HID, h] = att_src1[h]
        A1[h * HID:(h + 1) * HID, HEADS + h] = att_dst1[h]
    W1b = np.concatenate([W1, W1 @ A1], axis=1)          # [256, 264]
    A2 = np.zeros((OUT, 2), dtype=np.float32)
    A2[:, 0] = att_src2[0]
    A2[:, 1] = att_dst2[0]
    W2b = np.concatenate([W2, W2 @ A2], axis=1)          # [256, 42]

    # ---- host prep: graph schedule ----------------------------------------
    loops = np.arange(N, dtype=np.int64)
    src = np.concatenate([ei[0], loops])
    dst = np.concatenate([ei[1], loops])
    dbs, offs, totd, idx_arrs, node_of = _schedule(src, dst)

    # ---- L1: h1a = x @ W1b (node-sharded) ---------------------------------
    nc1 = _build_l1(mybir, bacc, tile, bass)
    W1b_packed = np.concatenate([W1b[0:P], W1b[P:2 * P]], axis=1)  # [128,528]
    in_maps1 = []
    for c in range(NCORES):
        xs = np.zeros((P, 2, NPAD), dtype=np.float32)
        xc = x[c * NPC:(c + 1) * NPC]                    # [6250, 256]
        xt = np.ascontiguousarray(xc.T)                  # [256, 6250]
        xs[:, 0, :NPC] = xt[0:P]
        xs[:, 1, :NPC] = xt[P:2 * P]
        in_maps1.append({"xT": xs, "W1b": W1b_packed})
    res1 = _run(nc1, in_maps1, trace=_profile is not None)
    if _profile is not None and res1.exec_time_ns:
        _profile.append(("L1", res1.exec_time_ns))

    # assemble full node table for layer-1 edge phase (fp16 row stream)
    table1 = np.zeros((N + 1, T1), dtype=np.float16)
    adst_all = np.zeros((N, HEADS), dtype=np.float32)
    for c in range(NCORES):
        slots = _slots(res1.results[c]["h1a"], C1)       # [NPAD, 264]
        table1[c * NPC:(c + 1) * NPC] = slots[:NPC, :T1].astype(np.float16)
        adst_all[c * NPC:(c + 1) * NPC] = slots[:NPC, T1:C1]
    table1[DUMMY, F1:T1] = BIG_NEG

    # ---- L2: layer-1 edge phase + ELU + dense -----------------------------
    nc2 = _build_l2(mybir, bacc, tile, bass, dbs, offs, totd)
    W2b_packed = (np.concatenate([W2b[0:P], W2b[P:2 * P]], axis=1)
                  .astype(np.float16))                   # [128, 84]
    bias1 = np.tile(b1.reshape(1, F1), (P, 1)).astype(np.float32)
    in_maps2 = []
    for c in range(NCORES):
        ad = np.zeros((P, NBLK * HEADS), dtype=np.float32)
        nof = node_of[c]
        for b in range(NBLK):
            valid = nof[b * P:(b + 1) * P]
            vm = valid >= 0
            ad_blk = np.zeros((P, HEADS), dtype=np.float32)
            ad_blk[vm] = adst_all[c * NPC + valid[vm]]
            ad[:, b * HEADS:(b + 1) * HEADS] = ad_blk
        in_maps2.append({"gst": table1[idx_arrs[c]], "adst": ad,
                         "bias": bias1, "W2b": W2b_packed})
    res2 = _run(nc2, in_maps2, trace=_profile is not None)
    if _profile is not None and res2.exec_time_ns:
        _profile.append(("L2", res2.exec_time_ns))

    # assemble layer-2 node table
    table2 = np.zeros((N + 1, T2), dtype=np.float16)
    adst2_pc = []
    for c in range(NCORES):
        h2pa = res2.results[c]["h2pa"]                   # [128, NBLK*42]
        slots = _slots(h2pa, C2)                         # [NPAD, 42]
        nof = node_of[c]
        vm = nof >= 0
        table2[c * NPC + nof[vm]] = slots[vm][:, :T2].astype(np.float16)
        adst2_pc.append(np.ascontiguousarray(h2pa[:, T2::C2]))  # [128, NBLK]
    table2[DUMMY, OUT:T2] = BIG_NEG

    # ---- L3: layer-2 edge phase + log_softmax -----------------------------
    nc3 = _build_l3(mybir, bacc, tile, bass, dbs, offs, totd)
    bias2 = np.tile(b2.reshape(1, OUT), (P, 1)).astype(np.float32)
    in_maps3 = []
    for c in range(NCORES):
        in_maps3.append({"gst": table2[idx_arrs[c]],
                         "adst": adst2_pc[c], "bias": bias2})
    res3 = _run(nc3, in_maps3, trace=_profile is not None)
    if _profile is not None and res3.exec_time_ns:
        _profile.append(("L3", res3.exec_time_ns))

    out = np.zeros((N, OUT), dtype=np.float32)
    for c in range(NCORES):
        slots = _slots(res3.results[c]["res"], OUT)      # [NPAD, 40]
        nof = node_of[c]
        vm = nof >= 0
        out[c * NPC + nof[vm]] = slots[vm]
    return out


# revision 13
# speedup vs baseline: 2.5912x; 1.8214x over previous
"""2-layer GAT (PyG GATConv-style, eval mode) on 8 Trainium2 NeuronCores.

Strategy (1D node partitioning, dst-sharded):
  - Nodes are sharded across 8 cores (6250 each). Three SPMD launches:
      L1: h1a = x @ [W1 | W1@A1]  (per-core node shard; A1 folds att vectors)
      L2: layer-1 edge phase (segment softmax-sum by dst) + bias + ELU
          + h2pa = elu1 @ [W2 | W2@A2]
      L3: layer-2 edge phase + bias + log_softmax
    The host concatenates per-core shards into full node tables between
    launches and pre-expands the per-edge-slot row streams (host-mediated
    halo exchange / gather; this bedrock image ships no HIPI GPSIMD ucode,
    so dma_gather / indirect DMA are unavailable on device).
  - Edge phase: per core, its dst nodes are sorted by in-degree (desc) and
    grouped into blocks of 128 (one dst node per partition). Each block's
    edges are padded to the block max degree d_b and laid out [128, d_b]
    along the free dimension as two fp16 streams: messages [128, d_b, F]
    and a_src [128, d_b, H]. a_dst is per-partition. Softmax uses a
    constant -10 logit shift (exact: softmax is shift-invariant; keeps
    exp() in fp16 range); denominators come from the same fp16 weights
    (+1e-20 so all-pad nodes stay finite). Pad slots carry h=0 and
    a_src=-120 so they contribute exactly 0.
  - Per block: exp() runs on the ScalarEngine writing a channel-expanded
    fp16 weight buffer (broadcast-read AP), the message scale is one
    unit-stride fp16 tensor_tensor (2x packed), and the segment sum runs
    on the otherwise-idle TensorEngine as d_b accumulating
    identity-matmuls into PSUM (fp32 accumulation).
"""

import numpy as np

N = 50000
E = 800000
D_IN = 256
HID = 64
HEADS = 4
OUT = 40
NEG_SLOPE = 0.2

NCORES = 8
NPC = N // NCORES          # 6250 nodes per core
P = 128
NBLK = (NPC + P - 1) // P  # 49 blocks per core
NPAD = NBLK * P            # 6272 slots per core
DUMMY = N                  # dummy row index in node tables
BIG_NEG = -120.0
SHIFT = 10.0               # constant logit shift before exp (fp16 range)

F1 = HEADS * HID           # 256
C1 = F1 + 2 * HEADS        # 264 = [h1 | a_src1 | a_dst1]
T1 = F1 + HEADS            # 260 table1 row = [h1 | a_src1]
C2 = OUT + 2               # 42  = [h2p | a_src2 | a_dst2]
T2 = OUT + 1               # 41 table2 row = [h2p | a_src2]


def _schedule(src, dst):
    """Per-core degree-sorted block schedule + gather index arrays."""
    core_of = dst // NPC
    dbs_core = np.zeros((NCORES, NBLK), dtype=np.int64)
    per_core = []
    for c in range(NCORES):
        m = core_of == c
        es = src[m].astype(np.int64)
        ed = (dst[m] - c * NPC).astype(np.int64)
        deg = np.bincount(ed, minlength=NPC)
        order = np.argsort(-deg, kind="stable")
        sidx = np.argsort(ed, kind="stable")
        es_sorted = es[sidx]
        starts = np.zeros(NPC + 1, dtype=np.int64)
        np.cumsum(deg, out=starts[1:])
        for b in range(NBLK):
            i = b * P
            dbs_core[c, b] = deg[order[i]] if i < NPC else 0
        per_core.append((deg, order, es_sorted, starts))

    dbs = np.maximum(dbs_core.max(axis=0), 1).astype(np.int64)
    offs = np.zeros(NBLK + 1, dtype=np.int64)
    np.cumsum(dbs, out=offs[1:])
    totd = int(offs[-1])

    idx_arrs = []
    node_of = []
    for c in range(NCORES):
        deg, order, es_sorted, starts = per_core[c]
        idx = np.full((P, totd), DUMMY, dtype=np.int32)
        nof = np.full(NPAD, -1, dtype=np.int64)
        for b in range(NBLK):
            o = offs[b]
            for p in range(P):
                i = b * P + p
                if i >= NPC:
                    break
                node = order[i]
                nof[i] = node
                d = deg[node]
                if d:
                    idx[p, o:o + d] = es_sorted[starts[node]:starts[node] + d]
        idx_arrs.append(idx)
        node_of.append(nof)
    return dbs, offs, totd, idx_arrs, node_of


def _slots(arr_128xnblkw, w):
    """[128, NBLK*w] core output -> [NPAD, w] slot-major rows."""
    return (
        arr_128xnblkw.reshape(P, NBLK, w).transpose(1, 0, 2).reshape(NPAD, w)
    )


def _build_l1(mybir, bacc, tile, bass):
    f32 = mybir.dt.float32
    f16 = mybir.dt.float16
    nc = bacc.Bacc("TRN2", target_bir_lowering=False, debug=False,
                   num_devices=NCORES)
    xT = nc.dram_tensor("xT", [P, 2, NPAD], f16, kind="ExternalInput")
    W1b = nc.dram_tensor("W1b", [P, 2 * C1], f16, kind="ExternalInput")
    h1a = nc.dram_tensor("h1a", [P, NBLK * C1], f32, kind="ExternalOutput")
    NCH = 7            # xT load chunks (7 blocks each)
    SGRP = 7           # output store batching
    with tile.TileContext(nc) as tc:
        with (
            tc.tile_pool(name="const", bufs=1) as cpool,
            tc.tile_pool(name="x", bufs=2) as xpool,
            tc.tile_pool(name="ps", bufs=4, space="PSUM") as pspool,
            tc.tile_pool(name="ev", bufs=2) as evpool,
        ):
            W1b_sb = cpool.tile([P, 2 * C1], f16)
            nc.sync.dma_start(out=W1b_sb[:], in_=W1b[:])
            nblk_per = NBLK // NCH
            assert nblk_per * NCH == NBLK
            for g in range(NCH):
                xt = xpool.tile([P, 2, nblk_per * P], f16, tag="xt")
                nc.sync.dma_start(
                    out=xt[:],
                    in_=xT[:, :, g * nblk_per * P:(g + 1) * nblk_per * P])
                ev = evpool.tile([P, nblk_per * C1], f32, tag="ev")
                for j in range(nblk_per):
                    ps = pspool.tile([P, C1], f32)
                    nc.tensor.matmul(ps[:], lhsT=xt[:, 0, j * P:(j + 1) * P],
                                     rhs=W1b_sb[:, 0:C1], start=True,
                                     stop=False)
                    nc.tensor.matmul(ps[:], lhsT=xt[:, 1, j * P:(j + 1) * P],
                                     rhs=W1b_sb[:, C1:2 * C1], start=False,
                                     stop=True)
                    if j % 2 == 0:
                        nc.vector.tensor_copy(ev[:, j * C1:(j + 1) * C1],
                                              ps[:])
                    else:
                        nc.scalar.copy(ev[:, j * C1:(j + 1) * C1], ps[:])
                b0 = g * nblk_per
                nc.sync.dma_start(
                    out=h1a[:, b0 * C1:(b0 + nblk_per) * C1], in_=ev[:])
    nc.compile()
    return nc


def _build_edge(mybir, bacc, tile, bass, dbs, offs, totd, *, nfeat, nhead,
                make_dense_tail, finish):
    """Edge-phase program builder shared by L2 and L3 (see module doc)."""
    f32 = mybir.dt.float32
    f16 = mybir.dt.float16
    ch = nfeat // nhead
    nc = bacc.Bacc("TRN2", target_bir_lowering=False, debug=False,
                   num_devices=NCORES)
    gmsg = nc.dram_tensor("gmsg", [P, totd, nfeat], f16, kind="ExternalInput")
    gsrc = nc.dram_tensor("gsrc", [P, totd * nhead], f16,
                          kind="ExternalInput")
    adst = nc.dram_tensor("adst", [P, NBLK * nhead], f32,
                          kind="ExternalInput")
    biast = nc.dram_tensor("bias", [P, nfeat], f32, kind="ExternalInput")

    from concourse.masks import make_identity

    with tile.TileContext(nc) as tc:
        with (
            tc.tile_pool(name="const", bufs=1) as cpool,
            tc.tile_pool(name="g", bufs=3) as gpool,
            tc.tile_pool(name="w", bufs=2) as wpool,
            tc.tile_pool(name="z", bufs=2) as zpool,
            tc.tile_pool(name="nsm", bufs=2) as npool,
            tc.tile_pool(name="ps", bufs=2, space="PSUM") as pspool,
        ):
            pools = (cpool, gpool, wpool, zpool, npool, pspool)
            gsrc_sb = cpool.tile([P, totd * nhead], f16)
            nc.sync.dma_start(out=gsrc_sb[:], in_=gsrc[:])
            adst_sb = cpool.tile([P, NBLK * nhead], f32)
            nc.sync.dma_start(out=adst_sb[:], in_=adst[:])
            bias_sb = cpool.tile([P, nfeat], f32)
            nc.sync.dma_start(out=bias_sb[:], in_=biast[:])
            shift_sb = cpool.tile([P, 1], f32)
            nc.vector.memset(shift_sb[:], -SHIFT)
            ident16 = cpool.tile([P, P], f16)
            make_identity(nc, ident16[:])
            tail_state = make_dense_tail(nc, tc, pools, None, None)

            for b in range(NBLK):
                db = int(dbs[b])
                o = int(offs[b])
                G = gpool.tile([P, db, nfeat], f16, tag="G")
                nc.sync.dma_start(out=G[:], in_=gmsg[:, o:o + db, :])
                # z = a_src[src] + a_dst[dst]; zr = leaky_relu(z)
                zl = zpool.tile([P, db, nhead], f32, tag="zl")
                a_view = (adst_sb[:, b * nhead:(b + 1) * nhead]
                          .unsqueeze(1).broadcast_to([P, db, nhead]))
                g_view = (gsrc_sb[:, o * nhead:(o + db) * nhead]
                          .rearrange("p (j h) -> p j h", h=nhead))
                nc.vector.tensor_tensor(zl[:], g_view, a_view,
                                        op=mybir.AluOpType.add)
                zr = zpool.tile([P, db, nhead], f32, tag="zr")
                nc.vector.scalar_tensor_tensor(
                    zr[:], in0=zl[:], scalar=NEG_SLOPE, in1=zl[:],
                    op0=mybir.AluOpType.mult, op1=mybir.AluOpType.max)
                # w = exp(zr - SHIFT), written channel-expanded in fp16 (ACT)
                w64 = wpool.tile([P, db, nhead, ch], f16, tag="w64")
                nc.scalar.activation(
                    w64[:], zr[:].unsqueeze(3).broadcast_to([P, db, nhead, ch]),
                    mybir.ActivationFunctionType.Exp, bias=shift_sb[:],
                    scale=1.0)
                # denominators from the same fp16 weights (+eps for pad rows)
                ws = npool.tile([P, nhead], f32, tag="ws")
                nc.vector.tensor_reduce(
                    ws[:], w64[:, :, :, 0].rearrange("p j h -> p h j"),
                    axis=mybir.AxisListType.X, op=mybir.AluOpType.add)
                nc.vector.tensor_scalar(ws[:], in0=ws[:], scalar1=1e-20,
                                        scalar2=None, op0=mybir.AluOpType.add)
                rws = npool.tile([P, nhead], f32, tag="rws")
                nc.vector.reciprocal(rws[:], ws[:])
                # scale messages in place (one unit-stride fp16 TT, 2x mode)
                gf = G[:].rearrange("p j c -> p (j c)")
                wf = w64[:].rearrange("p j h c -> p (j h c)")
                nc.vector.tensor_tensor(gf, gf, wf, op=mybir.AluOpType.mult)
                # segment sum on the TensorEngine: d_b accumulating
                # identity-matmuls into PSUM (fp32 accumulation)
                msum = pspool.tile([P, nfeat], f32, tag="msum")
                for j in range(db):
                    nc.tensor.matmul(msum[:], lhsT=ident16[:],
                                     rhs=G[:, j, :], start=(j == 0),
                                     stop=(j == db - 1))
                # normalize + bias
                o1 = npool.tile([P, nfeat], f32, tag="o1")
                nc.vector.tensor_tensor(
                    o1[:].rearrange("p (h c) -> p h c", h=nhead),
                    msum[:].rearrange("p (h c) -> p h c", h=nhead),
                    rws[:].unsqueeze(2).broadcast_to([P, nhead, ch]),
                    op=mybir.AluOpType.mult)
                o1b = npool.tile([P, nfeat], f32, tag="o1b")
                nc.vector.tensor_tensor(o1b[:], o1[:], bias_sb[:],
                                        op=mybir.AluOpType.add)
                make_dense_tail(nc, tc, pools, b, (o1b, tail_state))
            finish(nc, tc, pools, tail_state)
    nc.compile()
    return nc


def _build_l2(mybir, bacc, tile, bass, dbs, offs, totd):
    f32 = mybir.dt.float32
    f16 = mybir.dt.float16
    from concourse.masks import make_identity

    holder = {}

    def tail(nc, tc, pools, b, arg):
        cpool, gpool, wpool, zpool, npool, pspool = pools
        if b is None:
            W2b = nc.dram_tensor("W2b", [P, 2 * C2], f16,
                                 kind="ExternalInput")
            h2pa = nc.dram_tensor("h2pa", [P, NBLK * C2], f32,
                                  kind="ExternalOutput")
            W2b_sb = cpool.tile([P, 2 * C2], f16)
            nc.sync.dma_start(out=W2b_sb[:], in_=W2b[:])
            ident = cpool.tile([P, P], f16)
            make_identity(nc, ident[:])
            hacc = cpool.tile([P, NBLK * C2], f32)
            holder["st"] = (W2b_sb, ident, h2pa, hacc)
            return holder["st"]
        o1b, (W2b_sb, ident, h2pa, hacc) = arg
        # elu(x) = max(x, exp(min(x, 0)) - 1), emitted in fp16
        m0 = npool.tile([P, F1], f32, tag="m0")
        nc.vector.tensor_scalar(m0[:], in0=o1b[:], scalar1=0.0, scalar2=None,
                                op0=mybir.AluOpType.min)
        u = npool.tile([P, F1], f32, tag="u")
        nc.scalar.activation(u[:], m0[:], mybir.ActivationFunctionType.Exp)
        elu = npool.tile([P, F1], f16, tag="elu")
        nc.vector.scalar_tensor_tensor(
            elu[:], in0=u[:], scalar=-1.0, in1=o1b[:],
            op0=mybir.AluOpType.add, op1=mybir.AluOpType.max)
        # transpose elu -> [feat, node] for the dense tail matmul
        eT = []
        for k in range(2):
            psT = pspool.tile([P, P], f16, tag="psT")
            nc.tensor.transpose(psT[:], elu[:, k * P:(k + 1) * P], ident[:])
            eTk = npool.tile([P, P], f16, tag=f"eT{k}")
            nc.vector.tensor_copy(eTk[:], psT[:])
            eT.append(eTk)
        psC = pspool.tile([P, C2], f32, tag="psC")
        nc.tensor.matmul(psC[:], lhsT=eT[0][:], rhs=W2b_sb[:, 0:C2],
                         start=True, stop=False)
        nc.tensor.matmul(psC[:], lhsT=eT[1][:], rhs=W2b_sb[:, C2:2 * C2],
                         start=False, stop=True)
        nc.scalar.copy(hacc[:, b * C2:(b + 1) * C2], psC[:])

    def finish(nc, tc, pools, st):
        W2b_sb, ident, h2pa, hacc = st
        nc.sync.dma_start(out=h2pa[:], in_=hacc[:])

    return _build_edge(mybir, bacc, tile, bass, dbs, offs, totd,
                       nfeat=F1, nhead=HEADS, make_dense_tail=tail,
                       finish=finish)


def _build_l3(mybir, bacc, tile, bass, dbs, offs, totd):
    f32 = mybir.dt.float32
    holder = {}

    def tail(nc, tc, pools, b, arg):
        cpool, gpool, wpool, zpool, npool, pspool = pools
        if b is None:
            res = nc.dram_tensor("res", [P, NBLK * OUT], f32,
                                 kind="ExternalOutput")
            oacc = cpool.tile([P, NBLK * OUT], f32)
            holder["st"] = (res, oacc)
            return holder["st"]
        o2b, (res, oacc) = arg
        nc.vector.tensor_copy(oacc[:, b * OUT:(b + 1) * OUT], o2b[:])

    def finish(nc, tc, pools, st):
        # batched log_softmax over all 49 blocks (2 ACT table loads total)
        cpool, gpool, wpool, zpool, npool, pspool = pools
        res, oacc = st
        o3 = oacc[:].rearrange("p (b c) -> p b c", c=OUT)
        m = cpool.tile([P, NBLK], f32)
        nc.vector.tensor_reduce(m[:], o3, axis=mybir.AxisListType.X,
                                op=mybir.AluOpType.max)
        sh = cpool.tile([P, NBLK * OUT], f32)
        nc.vector.tensor_tensor(
            sh[:].rearrange("p (b c) -> p b c", c=OUT), o3,
            m[:].unsqueeze(2).broadcast_to([P, NBLK, OUT]),
            op=mybir.AluOpType.subtract)
        t = cpool.tile([P, NBLK * OUT], f32)
        nc.scalar.activation(t[:], sh[:], mybir.ActivationFunctionType.Exp)
        s = cpool.tile([P, NBLK], f32)
        nc.vector.tensor_reduce(s[:], t[:].rearrange("p (b c) -> p b c",
                                                     c=OUT),
                                axis=mybir.AxisListType.X,
                                op=mybir.AluOpType.add)
        ls = cpool.tile([P, NBLK], f32)
        nc.scalar.activation(ls[:], s[:], mybir.ActivationFunctionType.Ln)
        out_sb = cpool.tile([P, NBLK * OUT], f32)
        nc.vector.tensor_tensor(
            out_sb[:].rearrange("p (b c) -> p b c", c=OUT),
            sh[:].rearrange("p (b c) -> p b c", c=OUT),
            ls[:].unsqueeze(2).broadcast_to([P, NBLK, OUT]),
            op=mybir.AluOpType.subtract)
        nc.sync.dma_start(out=res[:], in_=out_sb[:])

    return _build_edge(mybir, bacc, tile, bass, dbs, offs, totd,
                       nfeat=OUT, nhead=1, make_dense_tail=tail,
                       finish=finish)


def _run(nc, in_maps, trace=False):
    from concourse import bass_utils
    return bass_utils.run_bass_kernel_spmd(
        nc, in_maps, core_ids=list(range(NCORES)), trace=trace)


def kernel(x, edge_index, W1, att_src1, att_dst1, b1, W2, att_src2, att_dst2,
           b2, _profile=None):
    import concourse.bacc as bacc
    import concourse.bass as bass
    import concourse.mybir as mybir
    import concourse.tile as tile

    x = np.asarray(x, dtype=np.float32)
    ei = np.asarray(edge_index, dtype=np.int64)
    W1 = np.asarray(W1, dtype=np.float32)
    att_src1 = np.asarray(att_src1, dtype=np.float32)
    att_dst1 = np.asarray(att_dst1, dtype=np.float32)
    b1 = np.asarray(b1, dtype=np.float32)
    W2 = np.asarray(W2, dtype=np.float32)
    att_src2 = np.asarray(att_src2, dtype=np.float32)
    att_dst2 = np.asarray(att_dst2, dtype=np.float32)
    b2 = np.asarray(b2, dtype=np.float32)

    # ---- host prep: weights ------------------------------------------------
    A1 = np.zeros((F1, 2 * HEADS), dtype=np.float32)
    for h in range(HEADS):
        A1[h * HID:(h + 1) * HID, h] = att_src1[h]
        A1[h * HID:(h + 1) * HID, HEADS + h] = att_dst1[h]
    W1b = np.concatenate([W1, W1 @ A1], axis=1)          # [256, 264]
    A2 = np.zeros((OUT, 2), dtype=np.float32)
    A2[:, 0] = att_src2[0]
    A2[:, 1] = att_dst2[0]
    W2b = np.concatenate([W2, W2 @ A2], axis=1)          # [256, 42]

    # ---- host prep: graph schedule ----------------------------------------
    loops = np.arange(N, dtype=np.int64)
    src = np.concatenate([ei[0], loops])
    dst = np.concatenate([ei[1], loops])
    dbs, offs, totd, idx_arrs, node_of = _schedule(src, dst)

    # ---- L1: h1a = x @ W1b (node-sharded) ---------------------------------
    nc1 = _build_l1(mybir, bacc, tile, bass)
    W1b_packed = (np.concatenate([W1b[0:P], W1b[P:2 * P]], axis=1)
                  .astype(np.float16))                   # [128, 528]
    in_maps1 = []
    for c in range(NCORES):
        xs = np.zeros((P, 2, NPAD), dtype=np.float16)
        xc = x[c * NPC:(c + 1) * NPC]                    # [6250, 256]
        xt = np.ascontiguousarray(xc.T).astype(np.float16)
        xs[:, 0, :NPC] = xt[0:P]
        xs[:, 1, :NPC] = xt[P:2 * P]
        in_maps1.append({"xT": xs, "W1b": W1b_packed})
    res1 = _run(nc1, in_maps1, trace=_profile is not None)
    if _profile is not None and res1.exec_time_ns:
        _profile.append(("L1", res1.exec_time_ns))

    # assemble full node tables for the layer-1 edge phase
    tmsg1 = np.zeros((N + 1, F1), dtype=np.float16)
    tsrc1 = np.zeros((N + 1, HEADS), dtype=np.float16)
    adst_all = np.zeros((N, HEADS), dtype=np.float32)
    for c in range(NCORES):
        slots = _slots(res1.results[c]["h1a"], C1)       # [NPAD, 264]
        tmsg1[c * NPC:(c + 1) * NPC] = slots[:NPC, :F1].astype(np.float16)
        tsrc1[c * NPC:(c + 1) * NPC] = slots[:NPC, F1:T1].astype(np.float16)
        adst_all[c * NPC:(c + 1) * NPC] = slots[:NPC, T1:C1]
    tsrc1[DUMMY] = BIG_NEG

    # ---- L2: layer-1 edge phase + ELU + dense -----------------------------
    nc2 = _build_l2(mybir, bacc, tile, bass, dbs, offs, totd)
    W2b_packed = (np.concatenate([W2b[0:P], W2b[P:2 * P]], axis=1)
                  .astype(np.float16))                   # [128, 84]
    bias1 = np.tile(b1.reshape(1, F1), (P, 1)).astype(np.float32)
    in_maps2 = []
    for c in range(NCORES):
        ad = np.zeros((P, NBLK * HEADS), dtype=np.float32)
        nof = node_of[c]
        for b in range(NBLK):
            valid = nof[b * P:(b + 1) * P]
            vm = valid >= 0
            ad_blk = np.zeros((P, HEADS), dtype=np.float32)
            ad_blk[vm] = adst_all[c * NPC + valid[vm]]
            ad[:, b * HEADS:(b + 1) * HEADS] = ad_blk
        in_maps2.append({"gmsg": tmsg1[idx_arrs[c]],
                         "gsrc": tsrc1[idx_arrs[c]].reshape(P, -1),
                         "adst": ad, "bias": bias1, "W2b": W2b_packed})
    res2 = _run(nc2, in_maps2, trace=_profile is not None)
    if _profile is not None and res2.exec_time_ns:
        _profile.append(("L2", res2.exec_time_ns))

    # assemble layer-2 node tables
    tmsg2 = np.zeros((N + 1, OUT), dtype=np.float16)
    tsrc2 = np.zeros((N + 1, 1), dtype=np.float16)
    adst2_pc = []
    for c in range(NCORES):
        h2pa = res2.results[c]["h2pa"]                   # [128, NBLK*42]
        slots = _slots(h2pa, C2)                         # [NPAD, 42]
        nof = node_of[c]
        vm = nof >= 0
        tmsg2[c * NPC + nof[vm]] = slots[vm][:, :OUT].astype(np.float16)
        tsrc2[c * NPC + nof[vm]] = slots[vm][:, OUT:T2].astype(np.float16)
        adst2_pc.append(np.ascontiguousarray(h2pa[:, T2::C2]))  # [128, NBLK]
    tsrc2[DUMMY] = BIG_NEG

    # ---- L3: layer-2 edge phase + log_softmax -----------------------------
    nc3 = _build_l3(mybir, bacc, tile, bass, dbs, offs, totd)
    bias2 = np.tile(b2.reshape(1, OUT), (P, 1)).astype(np.float32)
    in_maps3 = []
    for c in range(NCORES):
        in_maps3.append({"gmsg": tmsg2[idx_arrs[c]],
                         "gsrc": tsrc2[idx_arrs[c]].reshape(P, -1),
                         "adst": adst2_pc[c], "bias": bias2})
    res3 = _run(nc3, in_maps3, trace=_profile is not None)
    if _profile is not None and res3.exec_time_ns:
        _profile.append(("L3", res3.exec_time_ns))

    out = np.zeros((N, OUT), dtype=np.float32)
    for c in range(NCORES):
        slots = _slots(res3.results[c]["res"], OUT)      # [NPAD, 40]
        nof = node_of[c]
        vm = nof >= 0
        out[c * NPC + nof[vm]] = slots[vm]
    return out
